# revision 22
# baseline (speedup 1.0000x reference)
"""Trainium2 Bass kernel for nn_AlignmentMatrix.

Math (per batch b):
    out[b,i,j] = s_ctx[b,i] + s_asp[b,j] + (ctx[b]*w3) @ asp[b].T [i,j]
with ctx [B,L1,H]=[128,1024,600], asp [B,L2,H]=[128,128,600],
w_u=[w1;w2;w3] each [600].

Device-side formulation (PE does all O(L1*L2*H) work):
    outT[b,j,i] = s_ctx[b,i]*ones[j]                 (rank-1 PE matmul)
                + sum_d rhsp8[d,j] * ctx8[d,i]       (PE, 5 K-chunks of 120)
                + s_asp[b,j]                         (bias at PSUM->SBUF copy)
where ctx8 = e3m4(ctx), rhsp8 = e3m4(w3*asp) are cast on host, and
s_ctx = ctx@w1 (fp32 host math, shipped fp16, 32 KB/core) rides a K=1
matmul so the cross term needs no full-precision operand at all.
Measured rel err on the reference seed: 1.18e-2 (gate 2e-2).

Per-core HBM traffic: 11.1 MB fp8 reads + 4.2 MB fp16 writes.  The
read path of one NeuronCore saturates ~230-250 GB/s (engine-level;
more queues don't help), so wall time ~= read time and every
scheduling stall shows up 1:1 in the total:

- rhsp8 and ctx8 are PACKED into one dram row per (partition, batch)
  so each batch is ONE 0.69 MB DMA (5.76 KB/partition descriptors),
  alternating rings per batch.  Few DMAs -> no head-of-line waits on
  the 8 shared HWDGE completion-semaphore lanes (the per-slice version
  stalled a ring ~7 us at startup on exactly that).
- Batch 0 is split across both rings to unblock the first matmul ~1 us
  after the rings open.
- Output writes go SWDGE-only mid-run (a ring write's semaphore wait
  would head-of-line block that ring's later read issues); the final
  pair is split SWDGE/sync to shorten the tail.
- Dummy matmuls at t=0 lift the PE HAM clock gate (1.2 -> 2.4 GHz)
  before real work arrives; PSUM is 6-deep so copies never gate mms.

Sharding: data-parallel over batch, 16 batches per core across 8 cores.
"""

import numpy as np
import ml_dtypes

import concourse.bass as bass
import concourse.bacc as bacc
import concourse.mybir as mybir
import concourse.tile as tile
from concourse.bass_utils import run_bass_kernel_spmd

N_CORES = 8
B = 128
L1 = 1024  # ctx rows (i)
L2 = 128  # asp rows (j)
H = 600  # contraction dim (d)
BPC = B // N_CORES  # batches per core
KC = 5  # contraction chunks
KP = H // KC  # 120 rows per chunk
NI = 512  # moving free-dim per matmul (PSUM-bank bound for f32 out)
NIC = L1 // NI
OPACK = 2  # batches packed per output DMA
LOOKAHEAD = 6  # batches of prefetch in flight
N_WARM = 10  # dummy matmuls to lift the HAM clock gate during load ramp
N_SPLIT = 6  # early batches split across both rings (DMA ramp phase)
RW = KC * L2  # 640: rhsp8 bytes per (partition, batch) row
PB = RW + KC * L1  # 5760: packed row length

F32 = mybir.dt.float32
F16 = mybir.dt.float16
F8 = mybir.dt.float8e3  # e3m4: 4 mantissa bits, max 15.5
NP_F8 = ml_dtypes.float8_e3m4


def build_kernel():
    nc = bacc.Bacc(
        "TRN2", target_bir_lowering=False, debug=False, enable_asserts=False
    )
    pc8 = nc.dram_tensor("pc8", [KP, BPC, PB], F8, kind="ExternalInput").ap()
    sctx16 = nc.dram_tensor(
        "sctx16", [1, BPC * L1], F16, kind="ExternalInput"
    ).ap()
    saspT = nc.dram_tensor("saspT", [L2, BPC], F32, kind="ExternalInput").ap()
    outT = nc.dram_tensor(
        "outT", [BPC // OPACK, L2, OPACK, L1], F16, kind="ExternalOutput"
    ).ap()

    with tile.TileContext(nc) as tc:
        with (
            tc.tile_pool(name="consts", bufs=1) as consts,
            tc.tile_pool(name="pc_pool", bufs=LOOKAHEAD + 2) as pc_pool,
            tc.tile_pool(name="out_pool", bufs=4) as out_pool,
            tc.tile_pool(name="ps_out", bufs=6, space="PSUM") as ps_out,
            tc.tile_pool(name="ps_warm", bufs=1, space="PSUM") as ps_warm,
        ):
            # PE warmup: the HAM clock gate watches PE ARRAY activity and
            # needs ~3.4us of sustained work before it passes the full
            # 2.4 GHz clock.  The dummies must light up the whole array
            # (K=120, M=128) - a K=1 matmul is invisible to it.  Burn the
            # DMA ramp-up here so real matmuls start warm.
            warm_row = consts.tile([KP, L2 + NI], F16)
            nc.vector.memset(warm_row[:], 0.0)
            warm_ps = ps_warm.tile([L2, NI], F32)
            for _ in range(N_WARM):
                nc.tensor.matmul(
                    warm_ps[:],
                    warm_row[:, 0:L2],
                    warm_row[:, L2 : L2 + NI],
                    start=True,
                    stop=True,
                )

            ones_col = consts.tile([1, L2], F16)
            nc.gpsimd.memset(ones_col[:], 1.0)
            sasp_t = consts.tile([L2, BPC], F32)
            nc.sync.dma_start(sasp_t[:], saspT[:])
            sctx_t = consts.tile([1, BPC * L1], F16)
            nc.scalar.dma_start(sctx_t[:], sctx16[:])

            pc_tiles = {}

            def issue_loads(b):
                # One DMA per batch (rhsp8 | ctx8 packed), rings alternating
                # by batch.  The first four batches split across BOTH rings
                # so they arrive at ring-pair rate during the ramp - the PE
                # never idles >3.4us early on (which would re-throttle the
                # HAM clock gate and double the cost of every stall).
                ct = pc_pool.tile([KP, PB], F8, tag="pc")
                e0, e1 = (nc.sync, nc.scalar) if b % 2 == 0 else (nc.scalar, nc.sync)
                if b < N_SPLIT:
                    cut = RW + 2 * L1  # rank-1 + k0 + k1 ready after part 1
                    e0.dma_start(ct[:, 0:cut], pc8[:, b, 0:cut])
                    e1.dma_start(ct[:, cut:PB], pc8[:, b, cut:PB])
                else:
                    e0.dma_start(ct[:], pc8[:, b])
                pc_tiles[b] = ct

            for b in range(min(LOOKAHEAD, BPC)):
                issue_loads(b)

            out_sb = None
            for b in range(BPC):
                if b + LOOKAHEAD < BPC:
                    issue_loads(b + LOOKAHEAD)
                ct = pc_tiles.pop(b)
                if b % OPACK == 0:
                    out_sb = out_pool.tile([L2, OPACK, L1], F16, tag="out")
                # k-outer / c-inner: the batch's first matmuls need only the
                # first ring-half (rhsp+k0+k1), tolerating arrival skew
                # between the two rings during the ramp.
                pss = [
                    ps_out.tile([L2, NI], F32, tag="ps", name=f"ps{b}_{c}")
                    for c in range(NIC)
                ]
                for c in range(NIC):
                    # K=1 rank-1 seeds PSUM with s_ctx[i] (host-exact).
                    nc.tensor.matmul(
                        pss[c][:],
                        ones_col[:],
                        sctx_t[0:1, b * L1 + c * NI : b * L1 + (c + 1) * NI],
                        start=True,
                        stop=False,
                    )
                for k in range(KC):
                    for c in range(NIC):
                        nc.tensor.matmul(
                            pss[c][:],
                            ct[:, k * L2 : (k + 1) * L2],
                            ct[:, RW + k * L1 + c * NI : RW + k * L1 + (c + 1) * NI],
                            start=False,
                            stop=(k == KC - 1),
                        )
                for c in range(NIC):
                    # PSUM->SBUF copy folds in s_asp[j] as a per-partition
                    # bias; alternate engines so neither is the straggler.
                    dst = out_sb[:, b % OPACK, c * NI : (c + 1) * NI]
                    if c % 2 == 0:
                        nc.scalar.activation(
                            dst,
                            pss[c][:],
                            mybir.ActivationFunctionType.Identity,
                            bias=sasp_t[:, b : b + 1],
                            scale=1.0,
                        )
                    else:
                        nc.vector.tensor_scalar_add(
                            dst, pss[c][:], sasp_t[:, b : b + 1]
                        )
                g = b // OPACK
                if b == BPC - 1:
                    # Last batch: write each NI-chunk as its copy lands
                    # (sync has no copies to block; its reads are drained).
                    for c in range(NIC):
                        nc.sync.dma_start(
                            outT[g, :, 1:2, c * NI : (c + 1) * NI],
                            out_sb[:, 1:2, c * NI : (c + 1) * NI],
                        )
                elif b == BPC - 2:
                    nc.gpsimd.dma_start(outT[g, :, 0:1, :], out_sb[:, 0:1, :])
                elif b % OPACK == OPACK - 1:
                    nc.gpsimd.dma_start(outT[g], out_sb[:])

    nc.compile()
    return nc


_NC_CACHE = None


def _get_nc():
    global _NC_CACHE
    if _NC_CACHE is None:
        _NC_CACHE = build_kernel()
    return _NC_CACHE


def kernel(batch_size=None, ctx=None, asp=None, w_u=None, **run_kwargs):
    ctx = np.asarray(ctx, dtype=np.float32)
    asp = np.asarray(asp, dtype=np.float32)
    w_u = np.asarray(w_u, dtype=np.float32)
    w1 = w_u[:H, 0]
    w2 = w_u[H : 2 * H, 0]
    w3 = w_u[2 * H :, 0]

    # Host-side layout + dtype transforms (p-major, rhsp8|ctx8 packed so
    # each batch loads as one DMA with long per-partition rows).
    # ctx8[p, b, k, i] = e3m4(ctx[b, i, k*KP+p]); rhsp8 likewise from
    # w3*asp.  d = k*KP+p.
    ctx8 = np.ascontiguousarray(
        ctx.reshape(B, L1, KC, KP).transpose(3, 0, 2, 1)
    ).astype(NP_F8)
    rh = (asp * w3).reshape(B, L2, KC, KP).transpose(3, 0, 2, 1)
    rhsp8 = np.ascontiguousarray(rh).astype(NP_F8)
    pc8 = np.concatenate(
        [rhsp8.reshape(KP, B, RW), ctx8.reshape(KP, B, KC * L1)], axis=2
    )
    # s_ctx[b, i] = ctx@w1 (fp32), shipped fp16; s_asp[b, j] fp32.
    sctx = (ctx.reshape(B * L1, H) @ w1).reshape(B, L1).astype(np.float16)
    sasp = (asp.reshape(B * L2, H) @ w2).reshape(B, L2)

    nc = _get_nc()
    in_maps = [
        {
            "pc8": pc8[:, c * BPC : (c + 1) * BPC],
            "sctx16": sctx[c * BPC : (c + 1) * BPC].reshape(1, BPC * L1),
            "saspT": np.ascontiguousarray(sasp[c * BPC : (c + 1) * BPC].T),
        }
        for c in range(N_CORES)
    ]
    res = run_bass_kernel_spmd(
        nc, in_maps, core_ids=list(range(N_CORES)), **run_kwargs
    )
    outT = np.concatenate(
        [res.results[c]["outT"] for c in range(N_CORES)], axis=0
    ).astype(np.float32)  # [B//OPACK, L2, OPACK, L1]
    out = np.ascontiguousarray(
        outT.transpose(0, 2, 3, 1).reshape(B, L1, L2)
    )  # [B, L1, L2]
    if run_kwargs:
        return out, res
    return out


# revision 23
# speedup vs baseline: 1.0196x; 1.0196x over previous
"""Trainium2 Bass kernel for nn_AlignmentMatrix.

Math (per batch b):
    out[b,i,j] = s_ctx[b,i] + s_asp[b,j] + (ctx[b]*w3) @ asp[b].T [i,j]
with ctx [B,L1,H]=[128,1024,600], asp [B,L2,H]=[128,128,600],
w_u=[w1;w2;w3] each [600].

Device-side formulation (PE does all O(L1*L2*H) work):
    outT[b,j,i] = s_ctx[b,i]*ones[j]                 (rank-1 PE matmul)
                + sum_d rhsp8[d,j] * ctx8[d,i]       (PE, 5 K-chunks of 120)
                + s_asp[b,j]                         (bias at PSUM->SBUF copy)
where ctx8 = e3m4(ctx), rhsp8 = e3m4(w3*asp) are cast on host, and
s_ctx = ctx@w1 (fp32 host math, shipped fp16, 32 KB/core) rides a K=1
matmul so the cross term needs no full-precision operand at all.
Measured rel err on the reference seed: 1.18e-2 (gate 2e-2).

Per-core HBM traffic: 11.1 MB fp8 reads + 4.2 MB fp16 writes.  The
read path of one NeuronCore saturates ~230-250 GB/s (engine-level;
more queues don't help), so wall time ~= read time and every
scheduling stall shows up 1:1 in the total:

- rhsp8 and ctx8 are PACKED into one dram row per (partition, batch)
  so each batch is ONE 0.69 MB DMA (5.76 KB/partition descriptors),
  alternating rings per batch.  Few DMAs -> no head-of-line waits on
  the 8 shared HWDGE completion-semaphore lanes (the per-slice version
  stalled a ring ~7 us at startup on exactly that).
- Batch 0 is split across both rings to unblock the first matmul ~1 us
  after the rings open.
- Output writes go SWDGE-only mid-run (a ring write's semaphore wait
  would head-of-line block that ring's later read issues); the final
  pair is split SWDGE/sync to shorten the tail.
- Dummy matmuls at t=0 lift the PE HAM clock gate (1.2 -> 2.4 GHz)
  before real work arrives; PSUM is 6-deep so copies never gate mms.

Sharding: data-parallel over batch, 16 batches per core across 8 cores.
"""

import numpy as np
import ml_dtypes

import concourse.bass as bass
import concourse.bacc as bacc
import concourse.mybir as mybir
import concourse.tile as tile
from concourse.bass_utils import run_bass_kernel_spmd

N_CORES = 8
B = 128
L1 = 1024  # ctx rows (i)
L2 = 128  # asp rows (j)
H = 600  # contraction dim (d)
BPC = B // N_CORES  # batches per core
KC = 5  # contraction chunks
KP = H // KC  # 120 rows per chunk
NI = 512  # moving free-dim per matmul (PSUM-bank bound for f32 out)
NIC = L1 // NI
OPACK = 2  # batches packed per output DMA
LOOKAHEAD = 6  # batches of prefetch in flight
N_WARM = 10  # dummy matmuls to lift the HAM clock gate during load ramp
N_SPLIT = 6  # early batches split across both rings (DMA ramp phase)
RW = KC * L2  # 640: rhsp8 bytes per (partition, batch) row
PB = RW + KC * L1  # 5760: packed row length

F32 = mybir.dt.float32
F16 = mybir.dt.float16
F8 = mybir.dt.float8e3  # e3m4: 4 mantissa bits, max 15.5
NP_F8 = ml_dtypes.float8_e3m4


def build_kernel():
    nc = bacc.Bacc(
        "TRN2", target_bir_lowering=False, debug=False, enable_asserts=False
    )
    pc8 = nc.dram_tensor("pc8", [KP, BPC, PB], F8, kind="ExternalInput").ap()
    sctx16 = nc.dram_tensor(
        "sctx16", [1, BPC * L1], F16, kind="ExternalInput"
    ).ap()
    saspT = nc.dram_tensor("saspT", [L2, BPC], F32, kind="ExternalInput").ap()
    outT = nc.dram_tensor(
        "outT", [BPC // OPACK, L2, OPACK, L1], F16, kind="ExternalOutput"
    ).ap()

    with tile.TileContext(nc) as tc:
        with (
            tc.tile_pool(name="consts", bufs=1) as consts,
            tc.tile_pool(name="pc_pool", bufs=LOOKAHEAD + 2) as pc_pool,
            tc.tile_pool(name="out_pool", bufs=4) as out_pool,
            tc.tile_pool(name="ps_out", bufs=6, space="PSUM") as ps_out,
            tc.tile_pool(name="ps_warm", bufs=1, space="PSUM") as ps_warm,
        ):
            # PE warmup: the HAM clock gate watches PE ARRAY activity and
            # needs ~3.4us of sustained work before it passes the full
            # 2.4 GHz clock.  The dummies must light up the whole array
            # (K=120, M=128) - a K=1 matmul is invisible to it.  Burn the
            # DMA ramp-up here so real matmuls start warm.
            warm_row = consts.tile([KP, L2 + NI], F16)
            nc.vector.memset(warm_row[:], 0.0)
            warm_ps = ps_warm.tile([L2, NI], F32)
            for _ in range(N_WARM):
                nc.tensor.matmul(
                    warm_ps[:],
                    warm_row[:, 0:L2],
                    warm_row[:, L2 : L2 + NI],
                    start=True,
                    stop=True,
                )

            ones_col = consts.tile([1, L2], F16)
            nc.gpsimd.memset(ones_col[:], 1.0)
            sasp_t = consts.tile([L2, BPC], F32)
            nc.sync.dma_start(sasp_t[:], saspT[:])
            sctx_t = consts.tile([1, BPC * L1], F16)
            nc.scalar.dma_start(sctx_t[:], sctx16[:])

            pc_tiles = {}

            def issue_loads(b):
                # One DMA per batch (rhsp8 | ctx8 packed), rings alternating
                # by batch.  The first four batches split across BOTH rings
                # so they arrive at ring-pair rate during the ramp - the PE
                # never idles >3.4us early on (which would re-throttle the
                # HAM clock gate and double the cost of every stall).
                ct = pc_pool.tile([KP, PB], F8, tag="pc")
                e0, e1 = (nc.sync, nc.scalar) if b % 2 == 0 else (nc.scalar, nc.sync)
                if b < N_SPLIT or b >= BPC - 2:
                    # Ramp batches split across rings so neither ring lags
                    # the PE; tail batches split so both rings' reads end
                    # together (no straggler ring extending the drain).
                    cut = RW + 2 * L1  # rank-1 + k0 + k1 ready after part 1
                    e0.dma_start(ct[:, 0:cut], pc8[:, b, 0:cut])
                    e1.dma_start(ct[:, cut:PB], pc8[:, b, cut:PB])
                else:
                    e0.dma_start(ct[:], pc8[:, b])
                pc_tiles[b] = ct

            for b in range(min(LOOKAHEAD, BPC)):
                issue_loads(b)

            out_sb = None
            for b in range(BPC):
                if b + LOOKAHEAD < BPC:
                    issue_loads(b + LOOKAHEAD)
                ct = pc_tiles.pop(b)
                if b % OPACK == 0:
                    out_sb = out_pool.tile([L2, OPACK, L1], F16, tag="out")
                # k-outer / c-inner: the batch's first matmuls need only the
                # first ring-half (rhsp+k0+k1), tolerating arrival skew
                # between the two rings during the ramp.
                pss = [
                    ps_out.tile([L2, NI], F32, tag="ps", name=f"ps{b}_{c}")
                    for c in range(NIC)
                ]
                for c in range(NIC):
                    # K=1 rank-1 seeds PSUM with s_ctx[i] (host-exact).
                    nc.tensor.matmul(
                        pss[c][:],
                        ones_col[:],
                        sctx_t[0:1, b * L1 + c * NI : b * L1 + (c + 1) * NI],
                        start=True,
                        stop=False,
                    )
                for k in range(KC):
                    for c in range(NIC):
                        nc.tensor.matmul(
                            pss[c][:],
                            ct[:, k * L2 : (k + 1) * L2],
                            ct[:, RW + k * L1 + c * NI : RW + k * L1 + (c + 1) * NI],
                            start=False,
                            stop=(k == KC - 1),
                        )
                for c in range(NIC):
                    # PSUM->SBUF copy folds in s_asp[j] as a per-partition
                    # bias; alternate engines so neither is the straggler.
                    dst = out_sb[:, b % OPACK, c * NI : (c + 1) * NI]
                    if c % 2 == 0:
                        nc.scalar.activation(
                            dst,
                            pss[c][:],
                            mybir.ActivationFunctionType.Identity,
                            bias=sasp_t[:, b : b + 1],
                            scale=1.0,
                        )
                    else:
                        nc.vector.tensor_scalar_add(
                            dst, pss[c][:], sasp_t[:, b : b + 1]
                        )
                g = b // OPACK
                if b == BPC - 1:
                    # Last batch: write each NI-chunk as its copy lands
                    # (sync has no copies to block; its reads are drained).
                    for c in range(NIC):
                        nc.sync.dma_start(
                            outT[g, :, 1:2, c * NI : (c + 1) * NI],
                            out_sb[:, 1:2, c * NI : (c + 1) * NI],
                        )
                elif b == BPC - 2:
                    nc.gpsimd.dma_start(outT[g, :, 0:1, :], out_sb[:, 0:1, :])
                elif b % OPACK == OPACK - 1:
                    nc.gpsimd.dma_start(outT[g], out_sb[:])

    nc.compile()
    return nc


_NC_CACHE = None


def _get_nc():
    global _NC_CACHE
    if _NC_CACHE is None:
        _NC_CACHE = build_kernel()
    return _NC_CACHE


def kernel(batch_size=None, ctx=None, asp=None, w_u=None, **run_kwargs):
    ctx = np.asarray(ctx, dtype=np.float32)
    asp = np.asarray(asp, dtype=np.float32)
    w_u = np.asarray(w_u, dtype=np.float32)
    w1 = w_u[:H, 0]
    w2 = w_u[H : 2 * H, 0]
    w3 = w_u[2 * H :, 0]

    # Host-side layout + dtype transforms (p-major, rhsp8|ctx8 packed so
    # each batch loads as one DMA with long per-partition rows).
    # ctx8[p, b, k, i] = e3m4(ctx[b, i, k*KP+p]); rhsp8 likewise from
    # w3*asp.  d = k*KP+p.
    ctx8 = np.ascontiguousarray(
        ctx.reshape(B, L1, KC, KP).transpose(3, 0, 2, 1)
    ).astype(NP_F8)
    rh = (asp * w3).reshape(B, L2, KC, KP).transpose(3, 0, 2, 1)
    rhsp8 = np.ascontiguousarray(rh).astype(NP_F8)
    pc8 = np.concatenate(
        [rhsp8.reshape(KP, B, RW), ctx8.reshape(KP, B, KC * L1)], axis=2
    )
    # s_ctx[b, i] = ctx@w1 (fp32), shipped fp16; s_asp[b, j] fp32.
    sctx = (ctx.reshape(B * L1, H) @ w1).reshape(B, L1).astype(np.float16)
    sasp = (asp.reshape(B * L2, H) @ w2).reshape(B, L2)

    nc = _get_nc()
    in_maps = [
        {
            "pc8": pc8[:, c * BPC : (c + 1) * BPC],
            "sctx16": sctx[c * BPC : (c + 1) * BPC].reshape(1, BPC * L1),
            "saspT": np.ascontiguousarray(sasp[c * BPC : (c + 1) * BPC].T),
        }
        for c in range(N_CORES)
    ]
    res = run_bass_kernel_spmd(
        nc, in_maps, core_ids=list(range(N_CORES)), **run_kwargs
    )
    outT = np.concatenate(
        [res.results[c]["outT"] for c in range(N_CORES)], axis=0
    ).astype(np.float32)  # [B//OPACK, L2, OPACK, L1]
    out = np.ascontiguousarray(
        outT.transpose(0, 2, 3, 1).reshape(B, L1, L2)
    )  # [B, L1, L2]
    if run_kwargs:
        return out, res
    return out


# revision 33
# speedup vs baseline: 1.0985x; 1.0774x over previous
"""Trainium2 Bass kernel for nn_AlignmentMatrix.

Math (per batch b):
    out[b,i,j] = s_ctx[b,i] + s_asp[b,j] + (ctx[b]*w3) @ asp[b].T [i,j]
with ctx [B,L1,H]=[128,1024,600], asp [B,L2,H]=[128,128,600],
w_u=[w1;w2;w3] each [600].

Device-side formulation (PE does all O(L1*L2*H) work):
    outT[b,j,i] = s_ctx[b,i]*ones[j]                 (rank-1 PE matmul)
                + sum_d rhsp8[d,j] * ctx8[d,i]       (PE, 5 K-chunks of 120)
                + s_asp[b,j]                         (bias at PSUM->SBUF copy)
where ctx8 = e3m4(ctx), rhsp8 = e3m4(w3*asp) are cast on host, and
s_ctx = ctx@w1 (fp32 host math, shipped fp16, 32 KB/core) rides a K=1
matmul so the cross term needs no full-precision operand at all.
Measured rel err on the reference seed: 1.18e-2 (gate 2e-2).

Per-core HBM traffic: 11.1 MB fp8 reads + 4.2 MB fp16 writes.  The
read path of one NeuronCore saturates ~230-250 GB/s (engine-level;
more queues don't help), so wall time ~= read time and every
scheduling stall shows up 1:1 in the total:

- rhsp8 and ctx8 are PACKED into one dram row per (partition, batch)
  so each batch is ONE 0.69 MB DMA (5.76 KB/partition descriptors),
  alternating rings per batch.  Few DMAs -> no head-of-line waits on
  the 8 shared HWDGE completion-semaphore lanes (the per-slice version
  stalled a ring ~7 us at startup on exactly that).
- Batch 0 is split across both rings to unblock the first matmul ~1 us
  after the rings open.
- Output writes go SWDGE-only mid-run (a ring write's semaphore wait
  would head-of-line block that ring's later read issues); the final
  pair is split SWDGE/sync to shorten the tail.
- Dummy matmuls at t=0 lift the PE HAM clock gate (1.2 -> 2.4 GHz)
  before real work arrives; PSUM is 6-deep so copies never gate mms.

Sharding: data-parallel over batch, 16 batches per core across 8 cores.
"""

import numpy as np
import ml_dtypes

import concourse.bass as bass
import concourse.bacc as bacc
import concourse.mybir as mybir
import concourse.tile as tile
from concourse.bass_utils import run_bass_kernel_spmd

N_CORES = 8
B = 128
L1 = 1024  # ctx rows (i)
L2 = 128  # asp rows (j)
H = 600  # contraction dim (d)
BPC = B // N_CORES  # batches per core
KA = 128  # rows per main contraction chunk (full partition width)
KCA = 4  # main chunks (4 x 128 = 512 rows -> all 16 SDMA engines)
KB = H - KCA * KA  # 88: tail chunk rows
NI = 512  # moving free-dim per matmul (PSUM-bank bound for f32 out)
NIC = L1 // NI
OPACK = 2  # batches packed per output DMA
LOOKAHEAD = 6  # batches of prefetch in flight
N_WARM = 10  # dummy matmuls to lift the HAM clock gate during load ramp
N_SPLIT = 6  # early batches split across both rings (DMA ramp phase)
RWA = KCA * L2  # 512: main rhsp8 bytes per (partition, batch) row
PA = RWA + KCA * L1  # 4608: main packed row length
PBT = L2 + L1  # 1152: tail packed row length

F32 = mybir.dt.float32
F16 = mybir.dt.float16
F8 = mybir.dt.float8e3  # e3m4: 4 mantissa bits, max 15.5
NP_F8 = ml_dtypes.float8_e3m4


def build_kernel():
    nc = bacc.Bacc(
        "TRN2", target_bir_lowering=False, debug=False, enable_asserts=False
    )
    pa8 = nc.dram_tensor("pa8", [KA, BPC, PA], F8, kind="ExternalInput").ap()
    pb8 = nc.dram_tensor("pb8", [KB, BPC, PBT], F8, kind="ExternalInput").ap()
    sctx16 = nc.dram_tensor(
        "sctx16", [1, BPC * L1], F16, kind="ExternalInput"
    ).ap()
    saspT = nc.dram_tensor("saspT", [L2, BPC], F32, kind="ExternalInput").ap()
    outT = nc.dram_tensor(
        "outT", [BPC // OPACK, L2, OPACK, L1], F16, kind="ExternalOutput"
    ).ap()

    with tile.TileContext(nc) as tc:
        with (
            tc.tile_pool(name="consts", bufs=1) as consts,
            tc.tile_pool(name="pc_pool", bufs=LOOKAHEAD + 2) as pc_pool,
            tc.tile_pool(name="pcb_pool", bufs=LOOKAHEAD + 2) as pcb_pool,
            tc.tile_pool(name="out_pool", bufs=4) as out_pool,
            tc.tile_pool(name="ps_out", bufs=6, space="PSUM") as ps_out,
            tc.tile_pool(name="ps_warm", bufs=1, space="PSUM") as ps_warm,
        ):
            # PE warmup: the HAM clock gate watches PE ARRAY activity and
            # needs ~3.4us of sustained work before it passes the full
            # 2.4 GHz clock.  The dummies must light up the whole array
            # (K=120, M=128) - a K=1 matmul is invisible to it.  Burn the
            # DMA ramp-up here so real matmuls start warm.
            warm_row = consts.tile([KA, L2 + NI], F16)
            nc.vector.memset(warm_row[:], 0.0)
            warm_ps = ps_warm.tile([L2, NI], F32)
            for _ in range(N_WARM):
                nc.tensor.matmul(
                    warm_ps[:],
                    warm_row[:, 0:L2],
                    warm_row[:, L2 : L2 + NI],
                    start=True,
                    stop=True,
                )

            ones_col = consts.tile([1, L2], F16)
            nc.gpsimd.memset(ones_col[:], 1.0)
            sasp_t = consts.tile([L2, BPC], F32)
            nc.sync.dma_start(sasp_t[:], saspT[:])
            sctx_t = consts.tile([1, BPC * L1], F16)
            nc.scalar.dma_start(sctx_t[:], sctx16[:])

            pc_tiles = {}

            def issue_loads(b):
                # Main chunks (4x128 K-rows, all 16 SDMA engines) as one DMA
                # per batch on ring b%2; the 88-row tail chunk rides the
                # OTHER ring (per-pair byte balance + lockstep).  Ramp and
                # tail batches split the main row across both rings so no
                # ring lags the PE (an idle >3.4us re-throttles the HAM
                # clock gate and doubles the cost of every stall).
                ct = pc_pool.tile([KA, PA], F8, tag="pc")
                cb = pcb_pool.tile([KB, PBT], F8, tag="pcb")
                e0, e1 = (nc.sync, nc.scalar) if b % 2 == 0 else (nc.scalar, nc.sync)
                if b < N_SPLIT or b >= BPC - 2:
                    cut = RWA + 2 * L1  # rank-1 + k0 + k1 ready after part 1
                    e0.dma_start(ct[:, 0:cut], pa8[:, b, 0:cut])
                    e1.dma_start(ct[:, cut:PA], pa8[:, b, cut:PA])
                else:
                    e0.dma_start(ct[:], pa8[:, b])
                e1.dma_start(cb[:], pb8[:, b])
                pc_tiles[b] = (ct, cb)

            for b in range(min(LOOKAHEAD, BPC)):
                issue_loads(b)

            out_sb = None
            for b in range(BPC):
                if b + LOOKAHEAD < BPC:
                    issue_loads(b + LOOKAHEAD)
                ct, cb = pc_tiles.pop(b)
                if b % OPACK == 0:
                    out_sb = out_pool.tile([L2, OPACK, L1], F16, tag="out")
                # k-outer / c-inner: the batch's first matmuls need only the
                # first ring-half (rhsp+k0+k1), tolerating arrival skew
                # between the two rings during the ramp.
                pss = [
                    ps_out.tile([L2, NI], F32, tag="ps", name=f"ps{b}_{c}")
                    for c in range(NIC)
                ]
                for c in range(NIC):
                    # K=1 rank-1 seeds PSUM with s_ctx[i] (host-exact).
                    nc.tensor.matmul(
                        pss[c][:],
                        ones_col[:],
                        sctx_t[0:1, b * L1 + c * NI : b * L1 + (c + 1) * NI],
                        start=True,
                        stop=False,
                    )
                for k in range(KCA):
                    for c in range(NIC):
                        nc.tensor.matmul(
                            pss[c][:],
                            ct[:, k * L2 : (k + 1) * L2],
                            ct[:, RWA + k * L1 + c * NI : RWA + k * L1 + (c + 1) * NI],
                            start=False,
                            stop=False,
                        )
                for c in range(NIC):
                    nc.tensor.matmul(
                        pss[c][:],
                        cb[:, 0:L2],
                        cb[:, L2 + c * NI : L2 + (c + 1) * NI],
                        start=False,
                        stop=True,
                    )
                for c in range(NIC):
                    # PSUM->SBUF copy folds in s_asp[j] as a per-partition
                    # bias; alternate engines so neither is the straggler.
                    dst = out_sb[:, b % OPACK, c * NI : (c + 1) * NI]
                    if c % 2 == 0:
                        nc.scalar.activation(
                            dst,
                            pss[c][:],
                            mybir.ActivationFunctionType.Identity,
                            bias=sasp_t[:, b : b + 1],
                            scale=1.0,
                        )
                    else:
                        nc.vector.tensor_scalar_add(
                            dst, pss[c][:], sasp_t[:, b : b + 1]
                        )
                g = b // OPACK
                if b == BPC - 1:
                    # Last batch: write each NI-chunk as its copy lands, one
                    # per ring (reads are drained; sync has no copies to
                    # block and scalar's last copy precedes its write).
                    for c in range(NIC):
                        eng = nc.sync if c == 0 else nc.scalar
                        eng.dma_start(
                            outT[g, :, 1:2, c * NI : (c + 1) * NI],
                            out_sb[:, 1:2, c * NI : (c + 1) * NI],
                        )
                elif b == BPC - 2:
                    nc.gpsimd.dma_start(outT[g, :, 0:1, :], out_sb[:, 0:1, :])
                elif b % OPACK == OPACK - 1:
                    nc.gpsimd.dma_start(outT[g], out_sb[:])

    nc.compile()
    return nc


_NC_CACHE = None


def _get_nc():
    global _NC_CACHE
    if _NC_CACHE is None:
        _NC_CACHE = build_kernel()
    return _NC_CACHE


def kernel(batch_size=None, ctx=None, asp=None, w_u=None, **run_kwargs):
    ctx = np.asarray(ctx, dtype=np.float32)
    asp = np.asarray(asp, dtype=np.float32)
    w_u = np.asarray(w_u, dtype=np.float32)
    w1 = w_u[:H, 0]
    w2 = w_u[H : 2 * H, 0]
    w3 = w_u[2 * H :, 0]

    # Host-side layout + dtype transforms (p-major, rhsp8|ctx8 packed so
    # each batch loads as one DMA with long per-partition rows).  The
    # contraction dim splits 4x128 (main, fills all 128 partitions = all
    # 16 SDMA engines) + 88 (tail): d = k*128+p for k<4, d = 512+p after.
    rh = (asp * w3).astype(np.float32)
    HA = KCA * KA  # 512
    ctxA = np.ascontiguousarray(
        ctx[:, :, :HA].reshape(B, L1, KCA, KA).transpose(3, 0, 2, 1)
    ).astype(NP_F8)
    rhspA = np.ascontiguousarray(
        rh[:, :, :HA].reshape(B, L2, KCA, KA).transpose(3, 0, 2, 1)
    ).astype(NP_F8)
    pa8 = np.concatenate(
        [rhspA.reshape(KA, B, RWA), ctxA.reshape(KA, B, KCA * L1)], axis=2
    )
    ctxB = np.ascontiguousarray(ctx[:, :, HA:].transpose(2, 0, 1)).astype(NP_F8)
    rhspB = np.ascontiguousarray(rh[:, :, HA:].transpose(2, 0, 1)).astype(NP_F8)
    pb8 = np.concatenate([rhspB, ctxB], axis=2)
    # s_ctx[b, i] = ctx@w1 (fp32), shipped fp16; s_asp[b, j] fp32.
    sctx = (ctx.reshape(B * L1, H) @ w1).reshape(B, L1).astype(np.float16)
    sasp = (asp.reshape(B * L2, H) @ w2).reshape(B, L2)

    nc = _get_nc()
    in_maps = [
        {
            "pa8": pa8[:, c * BPC : (c + 1) * BPC],
            "pb8": pb8[:, c * BPC : (c + 1) * BPC],
            "sctx16": sctx[c * BPC : (c + 1) * BPC].reshape(1, BPC * L1),
            "saspT": np.ascontiguousarray(sasp[c * BPC : (c + 1) * BPC].T),
        }
        for c in range(N_CORES)
    ]
    res = run_bass_kernel_spmd(
        nc, in_maps, core_ids=list(range(N_CORES)), **run_kwargs
    )
    outT = np.concatenate(
        [res.results[c]["outT"] for c in range(N_CORES)], axis=0
    ).astype(np.float32)  # [B//OPACK, L2, OPACK, L1]
    out = np.ascontiguousarray(
        outT.transpose(0, 2, 3, 1).reshape(B, L1, L2)
    )  # [B, L1, L2]
    if run_kwargs:
        return out, res
    return out


# revision 34
# speedup vs baseline: 1.1090x; 1.0095x over previous
"""Trainium2 Bass kernel for nn_AlignmentMatrix.

Math (per batch b):
    out[b,i,j] = s_ctx[b,i] + s_asp[b,j] + (ctx[b]*w3) @ asp[b].T [i,j]
with ctx [B,L1,H]=[128,1024,600], asp [B,L2,H]=[128,128,600],
w_u=[w1;w2;w3] each [600].

Device-side formulation (PE does all O(L1*L2*H) work):
    outT[b,j,i] = s_ctx[b,i]*ones[j]                 (rank-1 PE matmul)
                + sum_d rhsp8[d,j] * ctx8[d,i]       (PE, 5 K-chunks of 120)
                + s_asp[b,j]                         (bias at PSUM->SBUF copy)
where ctx8 = e3m4(ctx), rhsp8 = e3m4(w3*asp) are cast on host, and
s_ctx = ctx@w1 (fp32 host math, shipped fp16, 32 KB/core) rides a K=1
matmul so the cross term needs no full-precision operand at all.
Measured rel err on the reference seed: 1.18e-2 (gate 2e-2).

Per-core HBM traffic: 11.1 MB fp8 reads + 4.2 MB fp16 writes.  The
read path of one NeuronCore saturates ~230-250 GB/s (engine-level;
more queues don't help), so wall time ~= read time and every
scheduling stall shows up 1:1 in the total:

- rhsp8 and ctx8 are PACKED into one dram row per (partition, batch)
  so each batch is ONE 0.69 MB DMA (5.76 KB/partition descriptors),
  alternating rings per batch.  Few DMAs -> no head-of-line waits on
  the 8 shared HWDGE completion-semaphore lanes (the per-slice version
  stalled a ring ~7 us at startup on exactly that).
- Batch 0 is split across both rings to unblock the first matmul ~1 us
  after the rings open.
- Output writes go SWDGE-only mid-run (a ring write's semaphore wait
  would head-of-line block that ring's later read issues); the final
  pair is split SWDGE/sync to shorten the tail.
- Dummy matmuls at t=0 lift the PE HAM clock gate (1.2 -> 2.4 GHz)
  before real work arrives; PSUM is 6-deep so copies never gate mms.

Sharding: data-parallel over batch, 16 batches per core across 8 cores.
"""

import numpy as np
import ml_dtypes

import concourse.bass as bass
import concourse.bacc as bacc
import concourse.mybir as mybir
import concourse.tile as tile
from concourse.bass_utils import run_bass_kernel_spmd

N_CORES = 8
B = 128
L1 = 1024  # ctx rows (i)
L2 = 128  # asp rows (j)
H = 600  # contraction dim (d)
BPC = B // N_CORES  # batches per core
KA = 128  # rows per main contraction chunk (full partition width)
KCA = 4  # main chunks (4 x 128 = 512 rows -> all 16 SDMA engines)
KB = H - KCA * KA  # 88: tail chunk rows
NI = 512  # moving free-dim per matmul (PSUM-bank bound for f32 out)
NIC = L1 // NI
OPACK = 2  # batches packed per output DMA
LOOKAHEAD = 6  # batches of prefetch in flight
N_WARM = 14  # dummy matmuls to lift the HAM clock gate during load ramp
N_SPLIT = 6  # early batches split across both rings (DMA ramp phase)
RWA = KCA * L2  # 512: main rhsp8 bytes per (partition, batch) row
PA = RWA + KCA * L1  # 4608: main packed row length
PBT = L2 + L1  # 1152: tail packed row length

F32 = mybir.dt.float32
F16 = mybir.dt.float16
F8 = mybir.dt.float8e3  # e3m4: 4 mantissa bits, max 15.5
NP_F8 = ml_dtypes.float8_e3m4


def build_kernel():
    nc = bacc.Bacc(
        "TRN2", target_bir_lowering=False, debug=False, enable_asserts=False
    )
    pa8 = nc.dram_tensor("pa8", [KA, BPC, PA], F8, kind="ExternalInput").ap()
    pb8 = nc.dram_tensor("pb8", [KB, BPC, PBT], F8, kind="ExternalInput").ap()
    sctx16 = nc.dram_tensor(
        "sctx16", [1, BPC * L1], F16, kind="ExternalInput"
    ).ap()
    saspT = nc.dram_tensor("saspT", [L2, BPC], F32, kind="ExternalInput").ap()
    outT = nc.dram_tensor(
        "outT", [BPC // OPACK, L2, OPACK, L1], F16, kind="ExternalOutput"
    ).ap()

    with tile.TileContext(nc) as tc:
        with (
            tc.tile_pool(name="consts", bufs=1) as consts,
            tc.tile_pool(name="pc_pool", bufs=LOOKAHEAD + 2) as pc_pool,
            tc.tile_pool(name="pcb_pool", bufs=LOOKAHEAD + 2) as pcb_pool,
            tc.tile_pool(name="out_pool", bufs=4) as out_pool,
            tc.tile_pool(name="ps_out", bufs=6, space="PSUM") as ps_out,
            tc.tile_pool(name="ps_warm", bufs=1, space="PSUM") as ps_warm,
        ):
            # PE warmup: the HAM clock gate watches PE ARRAY activity and
            # needs ~3.4us of sustained work before it passes the full
            # 2.4 GHz clock.  The dummies must light up the whole array
            # (K=120, M=128) - a K=1 matmul is invisible to it.  Burn the
            # DMA ramp-up here so real matmuls start warm.
            warm_row = consts.tile([KA, L2 + NI], F16)
            nc.vector.memset(warm_row[:], 0.0)
            warm_ps = ps_warm.tile([L2, NI], F32)
            for _ in range(N_WARM):
                nc.tensor.matmul(
                    warm_ps[:],
                    warm_row[:, 0:L2],
                    warm_row[:, L2 : L2 + NI],
                    start=True,
                    stop=True,
                )

            ones_col = consts.tile([1, L2], F16)
            nc.gpsimd.memset(ones_col[:], 1.0)
            sasp_t = consts.tile([L2, BPC], F32)
            nc.sync.dma_start(sasp_t[:], saspT[:])
            sctx_t = consts.tile([1, BPC * L1], F16)
            nc.scalar.dma_start(sctx_t[:], sctx16[:])

            pc_tiles = {}

            def issue_loads(b):
                # Main chunks (4x128 K-rows, all 16 SDMA engines) as one DMA
                # per batch on ring b%2; the 88-row tail chunk rides the
                # OTHER ring (per-pair byte balance + lockstep).  Ramp and
                # tail batches split the main row across both rings so no
                # ring lags the PE (an idle >3.4us re-throttles the HAM
                # clock gate and doubles the cost of every stall).
                ct = pc_pool.tile([KA, PA], F8, tag="pc")
                cb = pcb_pool.tile([KB, PBT], F8, tag="pcb")
                e0, e1 = (nc.sync, nc.scalar) if b % 2 == 0 else (nc.scalar, nc.sync)
                if b < N_SPLIT or b >= BPC - 2:
                    cut = RWA + 2 * L1  # rank-1 + k0 + k1 ready after part 1
                    e0.dma_start(ct[:, 0:cut], pa8[:, b, 0:cut])
                    e1.dma_start(ct[:, cut:PA], pa8[:, b, cut:PA])
                else:
                    e0.dma_start(ct[:], pa8[:, b])
                e1.dma_start(cb[:], pb8[:, b])
                pc_tiles[b] = (ct, cb)

            for b in range(min(LOOKAHEAD, BPC)):
                issue_loads(b)

            out_sb = None
            for b in range(BPC):
                if b + LOOKAHEAD < BPC:
                    issue_loads(b + LOOKAHEAD)
                ct, cb = pc_tiles.pop(b)
                if b % OPACK == 0:
                    out_sb = out_pool.tile([L2, OPACK, L1], F16, tag="out")
                # k-outer / c-inner: the batch's first matmuls need only the
                # first ring-half (rhsp+k0+k1), tolerating arrival skew
                # between the two rings during the ramp.
                pss = [
                    ps_out.tile([L2, NI], F32, tag="ps", name=f"ps{b}_{c}")
                    for c in range(NIC)
                ]
                for c in range(NIC):
                    # K=1 rank-1 seeds PSUM with s_ctx[i] (host-exact).
                    nc.tensor.matmul(
                        pss[c][:],
                        ones_col[:],
                        sctx_t[0:1, b * L1 + c * NI : b * L1 + (c + 1) * NI],
                        start=True,
                        stop=False,
                    )
                for k in range(KCA):
                    for c in range(NIC):
                        nc.tensor.matmul(
                            pss[c][:],
                            ct[:, k * L2 : (k + 1) * L2],
                            ct[:, RWA + k * L1 + c * NI : RWA + k * L1 + (c + 1) * NI],
                            start=False,
                            stop=False,
                        )
                for c in range(NIC):
                    nc.tensor.matmul(
                        pss[c][:],
                        cb[:, 0:L2],
                        cb[:, L2 + c * NI : L2 + (c + 1) * NI],
                        start=False,
                        stop=True,
                    )
                for c in range(NIC):
                    # PSUM->SBUF copy folds in s_asp[j] as a per-partition
                    # bias; alternate engines so neither is the straggler.
                    dst = out_sb[:, b % OPACK, c * NI : (c + 1) * NI]
                    if c % 2 == 0:
                        nc.scalar.activation(
                            dst,
                            pss[c][:],
                            mybir.ActivationFunctionType.Identity,
                            bias=sasp_t[:, b : b + 1],
                            scale=1.0,
                        )
                    else:
                        nc.vector.tensor_scalar_add(
                            dst, pss[c][:], sasp_t[:, b : b + 1]
                        )
                g = b // OPACK
                if b == BPC - 1:
                    # Last batch: write each NI-chunk as its copy lands, one
                    # per ring (reads are drained; sync has no copies to
                    # block and scalar's last copy precedes its write).
                    for c in range(NIC):
                        eng = nc.sync if c == 0 else nc.scalar
                        eng.dma_start(
                            outT[g, :, 1:2, c * NI : (c + 1) * NI],
                            out_sb[:, 1:2, c * NI : (c + 1) * NI],
                        )
                elif b == BPC - 2:
                    nc.gpsimd.dma_start(outT[g, :, 0:1, :], out_sb[:, 0:1, :])
                elif b % OPACK == OPACK - 1:
                    nc.gpsimd.dma_start(outT[g], out_sb[:])

    nc.compile()
    return nc


_NC_CACHE = None


def _get_nc():
    global _NC_CACHE
    if _NC_CACHE is None:
        _NC_CACHE = build_kernel()
    return _NC_CACHE


def kernel(batch_size=None, ctx=None, asp=None, w_u=None, **run_kwargs):
    ctx = np.asarray(ctx, dtype=np.float32)
    asp = np.asarray(asp, dtype=np.float32)
    w_u = np.asarray(w_u, dtype=np.float32)
    w1 = w_u[:H, 0]
    w2 = w_u[H : 2 * H, 0]
    w3 = w_u[2 * H :, 0]

    # Host-side layout + dtype transforms (p-major, rhsp8|ctx8 packed so
    # each batch loads as one DMA with long per-partition rows).  The
    # contraction dim splits 4x128 (main, fills all 128 partitions = all
    # 16 SDMA engines) + 88 (tail): d = k*128+p for k<4, d = 512+p after.
    rh = (asp * w3).astype(np.float32)
    HA = KCA * KA  # 512
    ctxA = np.ascontiguousarray(
        ctx[:, :, :HA].reshape(B, L1, KCA, KA).transpose(3, 0, 2, 1)
    ).astype(NP_F8)
    rhspA = np.ascontiguousarray(
        rh[:, :, :HA].reshape(B, L2, KCA, KA).transpose(3, 0, 2, 1)
    ).astype(NP_F8)
    pa8 = np.concatenate(
        [rhspA.reshape(KA, B, RWA), ctxA.reshape(KA, B, KCA * L1)], axis=2
    )
    ctxB = np.ascontiguousarray(ctx[:, :, HA:].transpose(2, 0, 1)).astype(NP_F8)
    rhspB = np.ascontiguousarray(rh[:, :, HA:].transpose(2, 0, 1)).astype(NP_F8)
    pb8 = np.concatenate([rhspB, ctxB], axis=2)
    # s_ctx[b, i] = ctx@w1 (fp32), shipped fp16; s_asp[b, j] fp32.
    sctx = (ctx.reshape(B * L1, H) @ w1).reshape(B, L1).astype(np.float16)
    sasp = (asp.reshape(B * L2, H) @ w2).reshape(B, L2)

    nc = _get_nc()
    in_maps = [
        {
            "pa8": pa8[:, c * BPC : (c + 1) * BPC],
            "pb8": pb8[:, c * BPC : (c + 1) * BPC],
            "sctx16": sctx[c * BPC : (c + 1) * BPC].reshape(1, BPC * L1),
            "saspT": np.ascontiguousarray(sasp[c * BPC : (c + 1) * BPC].T),
        }
        for c in range(N_CORES)
    ]
    res = run_bass_kernel_spmd(
        nc, in_maps, core_ids=list(range(N_CORES)), **run_kwargs
    )
    outT = np.concatenate(
        [res.results[c]["outT"] for c in range(N_CORES)], axis=0
    ).astype(np.float32)  # [B//OPACK, L2, OPACK, L1]
    out = np.ascontiguousarray(
        outT.transpose(0, 2, 3, 1).reshape(B, L1, L2)
    )  # [B, L1, L2]
    if run_kwargs:
        return out, res
    return out
